# revision 1
# baseline (speedup 1.0000x reference)
"""Self-contained Trainium2 Bass kernel for the routed-dense (MoE-style) layer.

Reference computation (per batch b, atom n):
    out[b,n,:] = tanh(W[ch[n]] @ x[b,n,:] + bias[ch[n]]) + x[b,n,:]
    returns (out, channels)

Strategy: route rows (b,n) by channel on the host; each of the 8 cores
processes 1/8 of every channel's rows, so every core runs the identical
program (SPMD) with perfect load balance.  On device everything lives in
transposed layout out^T[o, row] so the x^T tiles feed both the matmul
(contraction over IN on partitions) and the residual add (IN == OUT).
Matmuls run in float32r (full PE rate for moving dim >= 256).
"""

import sys

for _p in ("/opt/trn_rl_repo", "/root/.axon_site/_ro/trn_rl_repo"):
    if _p not in sys.path:
        sys.path.insert(0, _p)

import numpy as np

B, N, IN, OUT, C = 64, 1024, 512, 512, 8
NCORES = 8
P = 128
KC = IN // P   # 4 contraction chunks
OC = OUT // P  # 4 output-partition chunks

_cache = {}


def _plan_blocks(counts):
    """Per-channel row-chunking shared by every core.

    Returns a list of (channel, col_offset, block_size) where col_offset is
    the per-core column start and block sizes are <= 512, preferring >= 256
    (full fp32r rate).
    """
    blocks = []
    off = 0
    for c in range(C):
        rem = (counts[c] * B) // NCORES  # = 8 * counts[c]
        while rem > 0:
            if rem > 768:
                take = 512
            elif rem > 512:
                take = (rem // 2) + (rem & 1)  # split into two >=256 halves
            else:
                take = rem
            blocks.append((c, off, take))
            off += take
            rem -= take
    return blocks, off


def _build_program(counts):
    import concourse.bacc as bacc
    import concourse.tile as tile
    import concourse.bass as bass
    import concourse.mybir as mybir
    from contextlib import ExitStack

    DT = mybir.dt.float32
    F32R = mybir.dt.float32r

    blocks, rows = _plan_blocks(counts)
    assert rows == B * N // NCORES

    nc = bacc.Bacc("TRN2", target_bir_lowering=False, debug=False)
    xt_ext = nc.dram_tensor("xt", [IN, rows], DT, kind="ExternalInput")
    wt_ext = nc.dram_tensor("wt", [C, IN, OUT], DT, kind="ExternalInput")
    b_ext = nc.dram_tensor("b", [P, C * OC], DT, kind="ExternalInput")
    yt_ext = nc.dram_tensor("yt", [OUT, rows], DT, kind="ExternalOutput")

    with tile.TileContext(nc) as tc:
        with ExitStack() as ctx:
            wpool = ctx.enter_context(tc.tile_pool(name="w", bufs=1))
            bpool = ctx.enter_context(tc.tile_pool(name="bias", bufs=1))
            xpool = ctx.enter_context(tc.tile_pool(name="x", bufs=3))
            opool = ctx.enter_context(tc.tile_pool(name="o", bufs=8))
            ppool = ctx.enter_context(tc.tile_pool(name="p", bufs=8, space="PSUM"))

            # All C weights resident in SBUF: per channel [128, KC*OUT] f32r,
            # where chunk kc holds W^T[kc*128:(kc+1)*128, :] = [128, OUT].
            w_sb = []
            for c in range(C):
                w = wpool.tile([P, KC * OUT], F32R, tag=f"w{c}")
                for kc in range(KC):
                    nc.sync.dma_start(
                        w[:, bass.ts(kc, OUT)],
                        wt_ext[c, bass.ts(kc, P), :].bitcast(F32R),
                    )
                w_sb.append(w)
            b_sb = bpool.tile([P, C * OC], DT)
            nc.sync.dma_start(b_sb[:], b_ext[:])

            for ch, off, bs in blocks:
                xt = xpool.tile([P, KC * 512], F32R, tag="xt")
                for kc in range(KC):
                    nc.sync.dma_start(
                        xt[:, kc * 512 : kc * 512 + bs],
                        xt_ext[bass.ts(kc, P), off : off + bs].bitcast(F32R),
                    )
                for oc in range(OC):
                    psum = ppool.tile([P, 512], DT, tag="ps")
                    for kc in range(KC):
                        nc.tensor.matmul(
                            psum[:, :bs],
                            lhsT=w_sb[ch][:, kc * OUT + oc * P : kc * OUT + (oc + 1) * P],
                            rhs=xt[:, kc * 512 : kc * 512 + bs],
                            start=(kc == 0),
                            stop=(kc == KC - 1),
                        )
                    o_sb = opool.tile([P, 512], DT, tag="out")
                    nc.scalar.activation(
                        o_sb[:, :bs],
                        psum[:, :bs],
                        mybir.ActivationFunctionType.Tanh,
                        bias=b_sb[:, ch * OC + oc : ch * OC + oc + 1],
                    )
                    nc.vector.tensor_add(
                        o_sb[:, :bs],
                        o_sb[:, :bs],
                        xt[:, oc * 512 : oc * 512 + bs].bitcast(DT),
                    )
                    nc.sync.dma_start(
                        yt_ext[bass.ts(oc, P), off : off + bs], o_sb[:, :bs]
                    )

    nc.compile()
    return nc, blocks, rows


def _get_program(counts):
    key = tuple(int(c) for c in counts)
    if key not in _cache:
        _cache[key] = _build_program(counts)
    return _cache[key]


def kernel(x, channels, weight, bias, _want_trace=False):
    from concourse.bass_utils import run_bass_kernel_spmd

    x = np.asarray(x)
    ch_in = channels
    ch = np.asarray(channels).astype(np.int64)
    weight = np.asarray(weight, dtype=np.float32)
    bias = np.asarray(bias, dtype=np.float32)

    counts = np.bincount(ch, minlength=C)
    nc, blocks, rows = _get_program(counts)

    # ---- host-side routing / sharding ----
    perm = np.argsort(ch, kind="stable")           # atoms sorted by channel
    # global sorted row order: (sorted atom, batch), batch innermost
    sorted_rows = (perm[:, None] * B + np.arange(B)[None, :]).ravel()

    # per-core row ids: core k takes the k-th eighth of every channel's run
    run_len = counts * B
    run_start = np.concatenate([[0], np.cumsum(run_len)[:-1]])
    L = run_len // NCORES
    core_rows = np.empty((NCORES, rows), dtype=np.int64)
    for k in range(NCORES):
        parts = [
            sorted_rows[run_start[c] + k * L[c] : run_start[c] + (k + 1) * L[c]]
            for c in range(C)
        ]
        core_rows[k] = np.concatenate(parts)

    # x in [IN, N*B] layout, columns ordered (n, b) with b innermost
    xt_all = np.ascontiguousarray(x.transpose(2, 1, 0)).reshape(IN, N * B)

    wt = np.ascontiguousarray(weight.transpose(0, 2, 1))          # [C, IN, OUT]
    b_in = np.ascontiguousarray(
        bias.reshape(C, OC, P).transpose(2, 0, 1).reshape(P, C * OC)
    )

    in_maps = []
    for k in range(NCORES):
        xt_core = np.ascontiguousarray(xt_all[:, core_rows[k]])
        in_maps.append({"xt": xt_core, "wt": wt, "b": b_in})

    res = run_bass_kernel_spmd(
        nc, in_maps, list(range(NCORES)), trace=_want_trace
    )

    # ---- unshard ----
    out_nb = np.empty((N * B, OUT), dtype=np.float32)
    for k in range(NCORES):
        out_nb[core_rows[k]] = res.results[k]["yt"].T
    out = np.ascontiguousarray(
        out_nb.reshape(N, B, OUT).transpose(1, 0, 2)
    )

    if _want_trace:
        kernel._last_results = res
    return (out, ch_in)


# revision 2
# speedup vs baseline: 1.2306x; 1.2306x over previous
"""Self-contained Trainium2 Bass kernel for the routed-dense (MoE-style) layer.

Reference computation (per batch b, atom n):
    out[b,n,:] = tanh(W[ch[n]] @ x[b,n,:] + bias[ch[n]]) + x[b,n,:]
    returns (out, channels)

Strategy: expert-parallel — core c owns channel c outright (C == n_cores == 8),
so each core loads exactly one [OUT, IN] weight.  Rows (b, n) are routed to
cores by channel on the host; every core is padded to the max channel's row
count so all 8 cores run the identical SPMD program.  On device everything
lives in transposed layout out^T[o, row]: the x^T tiles feed both the matmul
(contraction over IN on partitions) and the residual add (IN == OUT).
Matmuls run in float32r (full PE rate for moving dim >= 256).  Host-side
x / out streams are block-packed so each block moves with a single DMA whose
per-partition runs are 4*block*4B contiguous.
"""

import sys

for _p in ("/opt/trn_rl_repo", "/root/.axon_site/_ro/trn_rl_repo"):
    if _p not in sys.path:
        sys.path.insert(0, _p)

import numpy as np

B, N, IN, OUT, C = 64, 1024, 512, 512, 8
NCORES = 8
P = 128
KC = IN // P   # 4 contraction chunks
OC = OUT // P  # 4 output-partition chunks

_cache = {}


def _plan_blocks(rows):
    """Chunk `rows` columns into blocks <= 512, preferring >= 256 so fp32r
    matmuls run at full rate."""
    blocks = []
    off = 0
    rem = rows
    while rem > 0:
        if rem > 768:
            take = 512
        elif rem > 512:
            take = (rem // 2) + (rem & 1)
        else:
            take = rem
        blocks.append((off, take))
        off += take
        rem -= take
    return blocks


def _build_program(rows):
    import concourse.bacc as bacc
    import concourse.tile as tile
    import concourse.mybir as mybir
    from contextlib import ExitStack

    DT = mybir.dt.float32
    F32R = mybir.dt.float32r

    blocks = _plan_blocks(rows)

    nc = bacc.Bacc("TRN2", target_bir_lowering=False, debug=False)
    # block-packed x^T: [128, 4 * rows], block b occupying [:, 4*off : 4*(off+bs)]
    # with inner layout [kc][col]
    xt_ext = nc.dram_tensor("xt", [P, KC * rows], DT, kind="ExternalInput")
    # weight^T for this core's channel, packed [128, KC * OUT] ([kc][o])
    wt_ext = nc.dram_tensor("wt", [P, KC * OUT], DT, kind="ExternalInput")
    b_ext = nc.dram_tensor("b", [P, OC], DT, kind="ExternalInput")
    # block-packed out^T: [128, 4 * rows], inner layout [oc][col]
    yt_ext = nc.dram_tensor("yt", [P, OC * rows], DT, kind="ExternalOutput")

    with tile.TileContext(nc) as tc:
        with ExitStack() as ctx:
            wpool = ctx.enter_context(tc.tile_pool(name="w", bufs=1))
            bpool = ctx.enter_context(tc.tile_pool(name="bias", bufs=1))
            xpool = ctx.enter_context(tc.tile_pool(name="x", bufs=4))
            opool = ctx.enter_context(tc.tile_pool(name="o", bufs=4))
            ppool = ctx.enter_context(tc.tile_pool(name="p", bufs=8, space="PSUM"))

            w_sb = wpool.tile([P, KC * OUT], F32R)
            nc.sync.dma_start(w_sb[:], wt_ext[:].bitcast(F32R))
            b_sb = bpool.tile([P, OC], DT)
            nc.sync.dma_start(b_sb[:], b_ext[:])

            for off, bs in blocks:
                xt = xpool.tile([P, KC * 512], F32R, tag="xt")
                nc.sync.dma_start(
                    xt[:, : KC * bs],
                    xt_ext[:, KC * off : KC * (off + bs)].bitcast(F32R),
                )
                o_sb = opool.tile([P, OC * 512], DT, tag="out")
                for oc in range(OC):
                    psum = ppool.tile([P, 512], DT, tag="ps")
                    for kc in range(KC):
                        nc.tensor.matmul(
                            psum[:, :bs],
                            lhsT=w_sb[:, kc * OUT + oc * P : kc * OUT + (oc + 1) * P],
                            rhs=xt[:, kc * bs : (kc + 1) * bs],
                            start=(kc == 0),
                            stop=(kc == KC - 1),
                        )
                    nc.scalar.activation(
                        o_sb[:, oc * bs : (oc + 1) * bs],
                        psum[:, :bs],
                        mybir.ActivationFunctionType.Tanh,
                        bias=b_sb[:, oc : oc + 1],
                    )
                    nc.vector.tensor_add(
                        o_sb[:, oc * bs : (oc + 1) * bs],
                        o_sb[:, oc * bs : (oc + 1) * bs],
                        xt[:, oc * bs : (oc + 1) * bs].bitcast(DT),
                    )
                nc.sync.dma_start(
                    yt_ext[:, OC * off : OC * (off + bs)], o_sb[:, : OC * bs]
                )

    nc.compile()
    return nc, blocks


def _get_program(rows):
    if rows not in _cache:
        _cache[rows] = _build_program(rows)
    return _cache[rows]


def kernel(x, channels, weight, bias, _want_trace=False):
    from concourse.bass_utils import run_bass_kernel_spmd

    x = np.asarray(x)
    ch_in = channels
    ch = np.asarray(channels).astype(np.int64)
    weight = np.asarray(weight, dtype=np.float32)
    bias = np.asarray(bias, dtype=np.float32)

    counts = np.bincount(ch, minlength=C)
    rows = int(counts.max()) * B
    nc, blocks = _get_program(rows)

    # ---- host-side routing / sharding ----
    # column order per core: this core's channel's atoms, batch innermost,
    # padded to `rows` by repeating the first column
    atom_ids = [np.where(ch == c)[0] for c in range(C)]
    x_nb = np.ascontiguousarray(x.transpose(2, 1, 0)).reshape(IN, N * B)

    core_cols = []
    for c in range(C):
        cols = (atom_ids[c][:, None] * B + np.arange(B)[None, :]).ravel()
        if len(cols) < rows:
            pad = np.zeros(rows - len(cols), dtype=np.int64)
            if len(cols):
                pad[:] = cols[0]
            cols = np.concatenate([cols, pad])
        core_cols.append(cols)

    in_maps = []
    for c in range(C):
        xt = x_nb[:, core_cols[c]]  # [IN, rows]
        # block-pack: [128, 4 * rows], block b -> [:, 4*off:4*(off+bs)] = [kc][col]
        xt_packed = np.empty((P, KC * rows), dtype=np.float32)
        xt4 = xt.reshape(KC, P, rows)
        for off, bs in blocks:
            xt_packed[:, KC * off : KC * (off + bs)] = (
                xt4[:, :, off : off + bs].transpose(1, 0, 2).reshape(P, KC * bs)
            )
        wt = np.ascontiguousarray(
            weight[c].T.reshape(KC, P, OUT).transpose(1, 0, 2).reshape(P, KC * OUT)
        )
        b_in = np.ascontiguousarray(bias[c].reshape(OC, P).T)
        in_maps.append({"xt": xt_packed, "wt": wt, "b": b_in})

    res = run_bass_kernel_spmd(
        nc, in_maps, list(range(NCORES)), trace=_want_trace
    )

    # ---- unshard ----
    out_nb = np.empty((N * B, OUT), dtype=np.float32)
    for c in range(C):
        yt_packed = res.results[c]["yt"]  # [128, OC * rows] block-packed
        n_real = len(atom_ids[c]) * B
        for off, bs in blocks:
            if off >= n_real:
                break
            take = min(bs, n_real - off)
            blk = (
                yt_packed[:, OC * off : OC * (off + bs)]
                .reshape(P, OC, bs)
                .transpose(1, 0, 2)
                .reshape(OUT, bs)
            )
            out_nb[core_cols[c][off : off + take]] = blk[:, :take].T
    out = np.ascontiguousarray(out_nb.reshape(N, B, OUT).transpose(1, 0, 2))

    if _want_trace:
        kernel._last_results = res
    return (out, ch_in)


# revision 3
# speedup vs baseline: 1.4612x; 1.1874x over previous
"""Self-contained Trainium2 Bass kernel for the routed-dense (MoE-style) layer.

Reference computation (per batch b, atom n):
    out[b,n,:] = tanh(W[ch[n]] @ x[b,n,:] + bias[ch[n]]) + x[b,n,:]
    returns (out, channels)

Strategy: expert-parallel — core c owns channel c outright (C == n_cores == 8),
so each core loads exactly one [OUT, IN] weight.  Rows (b, n) are routed to
cores by channel on the host; every core is padded to the max channel's row
count so all 8 cores run the identical SPMD program.  On device everything
lives in transposed layout out^T[o, row]: the x^T tiles feed both the matmul
(contraction over IN on partitions) and the residual add (IN == OUT).
Matmuls run in float32r (full PE rate for moving dim >= 256).  Host-side
x / out streams are block-packed so each block moves with a single DMA whose
per-partition runs are 4*block*4B contiguous.
"""

import sys

for _p in ("/opt/trn_rl_repo", "/root/.axon_site/_ro/trn_rl_repo"):
    if _p not in sys.path:
        sys.path.insert(0, _p)

import numpy as np

B, N, IN, OUT, C = 64, 1024, 512, 512, 8
NCORES = 8
P = 128
KC = IN // P   # 4 contraction chunks
OC = OUT // P  # 4 output-partition chunks

_cache = {}


def _plan_blocks(rows):
    """Chunk `rows` columns into blocks <= 512, preferring >= 256 so fp32r
    matmuls run at full rate."""
    blocks = []
    off = 0
    rem = rows
    while rem > 0:
        if rem > 768:
            take = 512
        elif rem > 512:
            take = (rem // 2) + (rem & 1)
        else:
            take = rem
        blocks.append((off, take))
        off += take
        rem -= take
    return blocks


def _build_program(rows):
    import concourse.bacc as bacc
    import concourse.tile as tile
    import concourse.mybir as mybir
    from contextlib import ExitStack

    DT = mybir.dt.float32
    F32R = mybir.dt.float32r

    blocks = _plan_blocks(rows)

    nc = bacc.Bacc("TRN2", target_bir_lowering=False, debug=False)
    # block-packed x^T: [128, 4 * rows], block b occupying [:, 4*off : 4*(off+bs)]
    # with inner layout [kc][col]
    xt_ext = nc.dram_tensor("xt", [P, KC * rows], DT, kind="ExternalInput")
    # weight^T for this core's channel, packed [128, KC * OUT] ([kc][o])
    wt_ext = nc.dram_tensor("wt", [P, KC * OUT], DT, kind="ExternalInput")
    b_ext = nc.dram_tensor("b", [P, OC], DT, kind="ExternalInput")
    # block-packed out^T: [128, 4 * rows], inner layout [oc][col]
    yt_ext = nc.dram_tensor("yt", [P, OC * rows], DT, kind="ExternalOutput")

    with tile.TileContext(nc) as tc:
        with ExitStack() as ctx:
            wpool = ctx.enter_context(tc.tile_pool(name="w", bufs=1))
            bpool = ctx.enter_context(tc.tile_pool(name="bias", bufs=1))
            xpool = ctx.enter_context(tc.tile_pool(name="x", bufs=6))
            opool = ctx.enter_context(tc.tile_pool(name="o", bufs=6))
            ppool = ctx.enter_context(tc.tile_pool(name="p", bufs=8, space="PSUM"))

            w_sb = wpool.tile([P, KC * OUT], F32R)
            nc.sync.dma_start(w_sb[:], wt_ext[:].bitcast(F32R))
            b_sb = bpool.tile([P, OC], DT)
            nc.sync.dma_start(b_sb[:], b_ext[:])

            for off, bs in blocks:
                xt = xpool.tile([P, KC * 512], F32R, tag="xt")
                nc.sync.dma_start(
                    xt[:, : KC * bs],
                    xt_ext[:, KC * off : KC * (off + bs)].bitcast(F32R),
                )
                o_sb = opool.tile([P, OC * 512], DT, tag="out")
                for oc in range(OC):
                    psum = ppool.tile([P, 512], DT, tag="ps")
                    for kc in range(KC):
                        nc.tensor.matmul(
                            psum[:, :bs],
                            lhsT=w_sb[:, kc * OUT + oc * P : kc * OUT + (oc + 1) * P],
                            rhs=xt[:, kc * bs : (kc + 1) * bs],
                            start=(kc == 0),
                            stop=(kc == KC - 1),
                        )
                    nc.scalar.activation(
                        o_sb[:, oc * bs : (oc + 1) * bs],
                        psum[:, :bs],
                        mybir.ActivationFunctionType.Tanh,
                        bias=b_sb[:, oc : oc + 1],
                    )
                    nc.vector.tensor_add(
                        o_sb[:, oc * bs : (oc + 1) * bs],
                        o_sb[:, oc * bs : (oc + 1) * bs],
                        xt[:, oc * bs : (oc + 1) * bs].bitcast(DT),
                    )
                nc.scalar.dma_start(
                    yt_ext[:, OC * off : OC * (off + bs)], o_sb[:, : OC * bs]
                )

    nc.compile()
    return nc, blocks


def _get_program(rows):
    if rows not in _cache:
        _cache[rows] = _build_program(rows)
    return _cache[rows]


def kernel(x, channels, weight, bias, _want_trace=False):
    from concourse.bass_utils import run_bass_kernel_spmd

    x = np.asarray(x)
    ch_in = channels
    ch = np.asarray(channels).astype(np.int64)
    weight = np.asarray(weight, dtype=np.float32)
    bias = np.asarray(bias, dtype=np.float32)

    counts = np.bincount(ch, minlength=C)
    rows = int(counts.max()) * B
    nc, blocks = _get_program(rows)

    # ---- host-side routing / sharding ----
    # column order per core: this core's channel's atoms, batch innermost,
    # padded to `rows` by repeating the first column
    atom_ids = [np.where(ch == c)[0] for c in range(C)]
    x_nb = np.ascontiguousarray(x.transpose(2, 1, 0)).reshape(IN, N * B)

    core_cols = []
    for c in range(C):
        cols = (atom_ids[c][:, None] * B + np.arange(B)[None, :]).ravel()
        if len(cols) < rows:
            pad = np.zeros(rows - len(cols), dtype=np.int64)
            if len(cols):
                pad[:] = cols[0]
            cols = np.concatenate([cols, pad])
        core_cols.append(cols)

    in_maps = []
    for c in range(C):
        xt = x_nb[:, core_cols[c]]  # [IN, rows]
        # block-pack: [128, 4 * rows], block b -> [:, 4*off:4*(off+bs)] = [kc][col]
        xt_packed = np.empty((P, KC * rows), dtype=np.float32)
        xt4 = xt.reshape(KC, P, rows)
        for off, bs in blocks:
            xt_packed[:, KC * off : KC * (off + bs)] = (
                xt4[:, :, off : off + bs].transpose(1, 0, 2).reshape(P, KC * bs)
            )
        wt = np.ascontiguousarray(
            weight[c].T.reshape(KC, P, OUT).transpose(1, 0, 2).reshape(P, KC * OUT)
        )
        b_in = np.ascontiguousarray(bias[c].reshape(OC, P).T)
        in_maps.append({"xt": xt_packed, "wt": wt, "b": b_in})

    res = run_bass_kernel_spmd(
        nc, in_maps, list(range(NCORES)), trace=_want_trace
    )

    # ---- unshard ----
    out_nb = np.empty((N * B, OUT), dtype=np.float32)
    for c in range(C):
        yt_packed = res.results[c]["yt"]  # [128, OC * rows] block-packed
        n_real = len(atom_ids[c]) * B
        for off, bs in blocks:
            if off >= n_real:
                break
            take = min(bs, n_real - off)
            blk = (
                yt_packed[:, OC * off : OC * (off + bs)]
                .reshape(P, OC, bs)
                .transpose(1, 0, 2)
                .reshape(OUT, bs)
            )
            out_nb[core_cols[c][off : off + take]] = blk[:, :take].T
    out = np.ascontiguousarray(out_nb.reshape(N, B, OUT).transpose(1, 0, 2))

    if _want_trace:
        kernel._last_results = res
    return (out, ch_in)


# revision 4
# speedup vs baseline: 1.8207x; 1.2460x over previous
"""Self-contained Trainium2 Bass kernel for the routed-dense (MoE-style) layer.

Reference computation (per batch b, atom n):
    out[b,n,:] = tanh(W[ch[n]] @ x[b,n,:] + bias[ch[n]]) + x[b,n,:]
    returns (out, channels)

Strategy: expert-parallel — core c owns channel c outright (C == n_cores == 8),
so each core loads exactly one [OUT, IN] weight.  Rows (b, n) are routed to
cores by channel on the host; every core is padded to the max channel's row
count so all 8 cores run the identical SPMD program.  On device everything
lives in transposed layout out^T[o, row]: the x^T tiles feed both the matmul
(contraction over IN on partitions) and the residual add (IN == OUT).
Matmuls run in float32r (full PE rate for moving dim >= 256).  Host-side
x / out streams are block-packed so each block moves with a single DMA whose
per-partition runs are 4*block*4B contiguous.
"""

import sys

for _p in ("/opt/trn_rl_repo", "/root/.axon_site/_ro/trn_rl_repo"):
    if _p not in sys.path:
        sys.path.insert(0, _p)

import numpy as np

B, N, IN, OUT, C = 64, 1024, 512, 512, 8
NCORES = 8
P = 128
KC = IN // P   # 4 contraction chunks
OC = OUT // P  # 4 output-partition chunks

_cache = {}


def _plan_blocks(rows):
    """Chunk `rows` columns into blocks <= 512, preferring >= 256 so fp32r
    matmuls run at full rate."""
    blocks = []
    off = 0
    rem = rows
    while rem > 0:
        if rem > 768:
            take = 512
        elif rem > 512:
            take = (rem // 2) + (rem & 1)
        else:
            take = rem
        blocks.append((off, take))
        off += take
        rem -= take
    return blocks


def _build_program(rows):
    import concourse.bacc as bacc
    import concourse.tile as tile
    import concourse.mybir as mybir
    from contextlib import ExitStack

    DT = mybir.dt.float32
    F16 = mybir.dt.float16
    F32R = mybir.dt.float32r

    blocks = _plan_blocks(rows)

    nc = bacc.Bacc("TRN2", target_bir_lowering=False, debug=False)
    # block-packed x^T: [128, 4 * rows], block b occupying [:, 4*off : 4*(off+bs)]
    # with inner layout [kc][col]
    xt_ext = nc.dram_tensor("xt", [P, KC * rows], DT, kind="ExternalInput")
    # weight^T for this core's channel, packed [128, KC * OUT] ([kc][o])
    wt_ext = nc.dram_tensor("wt", [P, KC * OUT], DT, kind="ExternalInput")
    b_ext = nc.dram_tensor("b", [P, OC], DT, kind="ExternalInput")
    # block-packed out^T: [128, 4 * rows], inner layout [oc][col]
    yt_ext = nc.dram_tensor("yt", [P, OC * rows], F16, kind="ExternalOutput")

    with tile.TileContext(nc) as tc:
        with ExitStack() as ctx:
            wpool = ctx.enter_context(tc.tile_pool(name="w", bufs=1))
            bpool = ctx.enter_context(tc.tile_pool(name="bias", bufs=1))
            xpool = ctx.enter_context(tc.tile_pool(name="x", bufs=6))
            apool = ctx.enter_context(tc.tile_pool(name="a", bufs=8))
            opool = ctx.enter_context(tc.tile_pool(name="o", bufs=6))
            ppool = ctx.enter_context(tc.tile_pool(name="p", bufs=8, space="PSUM"))

            w_sb = wpool.tile([P, KC * OUT], F32R)
            nc.sync.dma_start(w_sb[:], wt_ext[:].bitcast(F32R))
            b_sb = bpool.tile([P, OC], DT)
            nc.sync.dma_start(b_sb[:], b_ext[:])

            for off, bs in blocks:
                xt = xpool.tile([P, KC * 512], F32R, tag="xt")
                nc.sync.dma_start(
                    xt[:, : KC * bs],
                    xt_ext[:, KC * off : KC * (off + bs)].bitcast(F32R),
                )
                o_sb = opool.tile([P, OC * 512], F16, tag="out")
                for oc in range(OC):
                    psum = ppool.tile([P, 512], DT, tag="ps")
                    for kc in range(KC):
                        nc.tensor.matmul(
                            psum[:, :bs],
                            lhsT=w_sb[:, kc * OUT + oc * P : kc * OUT + (oc + 1) * P],
                            rhs=xt[:, kc * bs : (kc + 1) * bs],
                            start=(kc == 0),
                            stop=(kc == KC - 1),
                        )
                    a_sb = apool.tile([P, 512], DT, tag="act")
                    nc.scalar.activation(
                        a_sb[:, :bs],
                        psum[:, :bs],
                        mybir.ActivationFunctionType.Tanh,
                        bias=b_sb[:, oc : oc + 1],
                    )
                    nc.vector.tensor_add(
                        o_sb[:, oc * bs : (oc + 1) * bs],
                        a_sb[:, :bs],
                        xt[:, oc * bs : (oc + 1) * bs].bitcast(DT),
                    )
                nc.scalar.dma_start(
                    yt_ext[:, OC * off : OC * (off + bs)], o_sb[:, : OC * bs]
                )

    nc.compile()
    return nc, blocks


def _get_program(rows):
    if rows not in _cache:
        _cache[rows] = _build_program(rows)
    return _cache[rows]


def kernel(x, channels, weight, bias, _want_trace=False):
    from concourse.bass_utils import run_bass_kernel_spmd

    x = np.asarray(x)
    ch_in = channels
    ch = np.asarray(channels).astype(np.int64)
    weight = np.asarray(weight, dtype=np.float32)
    bias = np.asarray(bias, dtype=np.float32)

    counts = np.bincount(ch, minlength=C)
    rows = int(counts.max()) * B
    nc, blocks = _get_program(rows)

    # ---- host-side routing / sharding ----
    # column order per core: this core's channel's atoms, batch innermost,
    # padded to `rows` by repeating the first column
    atom_ids = [np.where(ch == c)[0] for c in range(C)]
    x_nb = np.ascontiguousarray(x.transpose(2, 1, 0)).reshape(IN, N * B)

    core_cols = []
    for c in range(C):
        cols = (atom_ids[c][:, None] * B + np.arange(B)[None, :]).ravel()
        if len(cols) < rows:
            pad = np.zeros(rows - len(cols), dtype=np.int64)
            if len(cols):
                pad[:] = cols[0]
            cols = np.concatenate([cols, pad])
        core_cols.append(cols)

    in_maps = []
    for c in range(C):
        xt = x_nb[:, core_cols[c]]  # [IN, rows]
        # block-pack: [128, 4 * rows], block b -> [:, 4*off:4*(off+bs)] = [kc][col]
        xt_packed = np.empty((P, KC * rows), dtype=np.float32)
        xt4 = xt.reshape(KC, P, rows)
        for off, bs in blocks:
            xt_packed[:, KC * off : KC * (off + bs)] = (
                xt4[:, :, off : off + bs].transpose(1, 0, 2).reshape(P, KC * bs)
            )
        wt = np.ascontiguousarray(
            weight[c].T.reshape(KC, P, OUT).transpose(1, 0, 2).reshape(P, KC * OUT)
        )
        b_in = np.ascontiguousarray(bias[c].reshape(OC, P).T)
        in_maps.append({"xt": xt_packed, "wt": wt, "b": b_in})

    res = run_bass_kernel_spmd(
        nc, in_maps, list(range(NCORES)), trace=_want_trace
    )

    # ---- unshard ----
    out_nb = np.empty((N * B, OUT), dtype=np.float32)
    for c in range(C):
        yt_packed = res.results[c]["yt"].astype(np.float32)  # [128, OC*rows] packed
        n_real = len(atom_ids[c]) * B
        for off, bs in blocks:
            if off >= n_real:
                break
            take = min(bs, n_real - off)
            blk = (
                yt_packed[:, OC * off : OC * (off + bs)]
                .reshape(P, OC, bs)
                .transpose(1, 0, 2)
                .reshape(OUT, bs)
            )
            out_nb[core_cols[c][off : off + take]] = blk[:, :take].T
    out = np.ascontiguousarray(out_nb.reshape(N, B, OUT).transpose(1, 0, 2))

    if _want_trace:
        kernel._last_results = res
    return (out, ch_in)


# revision 5
# speedup vs baseline: 1.9025x; 1.0449x over previous
"""Self-contained Trainium2 Bass kernel for the routed-dense (MoE-style) layer.

Reference computation (per batch b, atom n):
    out[b,n,:] = tanh(W[ch[n]] @ x[b,n,:] + bias[ch[n]]) + x[b,n,:]
    returns (out, channels)

Strategy: expert-parallel — core c owns channel c outright (C == n_cores == 8),
so each core loads exactly one [OUT, IN] weight.  Rows (b, n) are routed to
cores by channel on the host; every core is padded to the max channel's row
count so all 8 cores run the identical SPMD program.  On device everything
lives in transposed layout out^T[o, row]: the x^T tiles feed both the matmul
(contraction over IN on partitions) and the residual add (IN == OUT).
Matmuls run in float32r (full PE rate for moving dim >= 256).  Host-side
x / out streams are block-packed so each block moves with a single DMA whose
per-partition runs are 4*block*4B contiguous.
"""

import sys

for _p in ("/opt/trn_rl_repo", "/root/.axon_site/_ro/trn_rl_repo"):
    if _p not in sys.path:
        sys.path.insert(0, _p)

import numpy as np

B, N, IN, OUT, C = 64, 1024, 512, 512, 8
NCORES = 8
P = 128
KC = IN // P   # 4 contraction chunks
OC = OUT // P  # 4 output-partition chunks

_cache = {}


def _plan_blocks(rows):
    """Chunk `rows` columns into blocks <= 512, preferring >= 256 so fp32r
    matmuls run at full rate."""
    blocks = []
    off = 0
    rem = rows
    while rem > 0:
        if rem > 768:
            take = 512
        elif rem > 512:
            take = (rem // 2) + (rem & 1)
        else:
            take = rem
        blocks.append((off, take))
        off += take
        rem -= take
    return blocks


def _build_program(rows):
    import concourse.bacc as bacc
    import concourse.tile as tile
    import concourse.mybir as mybir
    from contextlib import ExitStack

    DT = mybir.dt.float32
    F16 = mybir.dt.float16
    F32R = mybir.dt.float32r

    blocks = _plan_blocks(rows)

    nc = bacc.Bacc("TRN2", target_bir_lowering=False, debug=False)
    # block-packed x^T: [128, 4 * rows], block b occupying [:, 4*off : 4*(off+bs)]
    # with inner layout [kc][col]
    xt_ext = nc.dram_tensor("xt", [P, KC * rows], DT, kind="ExternalInput")
    # weight^T for this core's channel, packed [128, KC * OUT] ([kc][o])
    wt_ext = nc.dram_tensor("wt", [P, KC * OUT], DT, kind="ExternalInput")
    b_ext = nc.dram_tensor("b", [P, OC], DT, kind="ExternalInput")
    # block-packed out^T: [128, 4 * rows], inner layout [oc][col]
    yt_ext = nc.dram_tensor("yt", [P, OC * rows], F16, kind="ExternalOutput")

    with tile.TileContext(nc) as tc:
        with ExitStack() as ctx:
            wpool = ctx.enter_context(tc.tile_pool(name="w", bufs=1))
            bpool = ctx.enter_context(tc.tile_pool(name="bias", bufs=1))
            xpool = ctx.enter_context(tc.tile_pool(name="x", bufs=9))
            apool = ctx.enter_context(tc.tile_pool(name="a", bufs=8))
            opool = ctx.enter_context(tc.tile_pool(name="o", bufs=8))
            ppool = ctx.enter_context(tc.tile_pool(name="p", bufs=8, space="PSUM"))

            w_sb = wpool.tile([P, KC * OUT], F32R)
            nc.sync.dma_start(w_sb[:], wt_ext[:].bitcast(F32R))
            b_sb = bpool.tile([P, OC], DT)
            nc.sync.dma_start(b_sb[:], b_ext[:])

            for off, bs in blocks:
                xt = xpool.tile([P, KC * 512], F32R, tag="xt")
                nc.sync.dma_start(
                    xt[:, : KC * bs],
                    xt_ext[:, KC * off : KC * (off + bs)].bitcast(F32R),
                )
                o_sb = opool.tile([P, OC * 512], F16, tag="out")
                for oc in range(OC):
                    psum = ppool.tile([P, 512], DT, tag="ps")
                    for kc in range(KC):
                        nc.tensor.matmul(
                            psum[:, :bs],
                            lhsT=w_sb[:, kc * OUT + oc * P : kc * OUT + (oc + 1) * P],
                            rhs=xt[:, kc * bs : (kc + 1) * bs],
                            start=(kc == 0),
                            stop=(kc == KC - 1),
                        )
                    a_sb = apool.tile([P, 512], DT, tag="act")
                    nc.scalar.activation(
                        a_sb[:, :bs],
                        psum[:, :bs],
                        mybir.ActivationFunctionType.Tanh,
                        bias=b_sb[:, oc : oc + 1],
                    )
                    nc.vector.tensor_add(
                        o_sb[:, oc * bs : (oc + 1) * bs],
                        a_sb[:, :bs],
                        xt[:, oc * bs : (oc + 1) * bs].bitcast(DT),
                    )
                nc.scalar.dma_start(
                    yt_ext[:, OC * off : OC * (off + bs)], o_sb[:, : OC * bs]
                )

    nc.compile()
    return nc, blocks


def _get_program(rows):
    if rows not in _cache:
        _cache[rows] = _build_program(rows)
    return _cache[rows]


def kernel(x, channels, weight, bias, _want_trace=False):
    from concourse.bass_utils import run_bass_kernel_spmd

    x = np.asarray(x)
    ch_in = channels
    ch = np.asarray(channels).astype(np.int64)
    weight = np.asarray(weight, dtype=np.float32)
    bias = np.asarray(bias, dtype=np.float32)

    counts = np.bincount(ch, minlength=C)
    rows = int(counts.max()) * B
    nc, blocks = _get_program(rows)

    # ---- host-side routing / sharding ----
    # column order per core: this core's channel's atoms, batch innermost,
    # padded to `rows` by repeating the first column
    atom_ids = [np.where(ch == c)[0] for c in range(C)]
    x_nb = np.ascontiguousarray(x.transpose(2, 1, 0)).reshape(IN, N * B)

    core_cols = []
    for c in range(C):
        cols = (atom_ids[c][:, None] * B + np.arange(B)[None, :]).ravel()
        if len(cols) < rows:
            pad = np.zeros(rows - len(cols), dtype=np.int64)
            if len(cols):
                pad[:] = cols[0]
            cols = np.concatenate([cols, pad])
        core_cols.append(cols)

    in_maps = []
    for c in range(C):
        xt = x_nb[:, core_cols[c]]  # [IN, rows]
        # block-pack: [128, 4 * rows], block b -> [:, 4*off:4*(off+bs)] = [kc][col]
        xt_packed = np.empty((P, KC * rows), dtype=np.float32)
        xt4 = xt.reshape(KC, P, rows)
        for off, bs in blocks:
            xt_packed[:, KC * off : KC * (off + bs)] = (
                xt4[:, :, off : off + bs].transpose(1, 0, 2).reshape(P, KC * bs)
            )
        wt = np.ascontiguousarray(
            weight[c].T.reshape(KC, P, OUT).transpose(1, 0, 2).reshape(P, KC * OUT)
        )
        b_in = np.ascontiguousarray(bias[c].reshape(OC, P).T)
        in_maps.append({"xt": xt_packed, "wt": wt, "b": b_in})

    res = run_bass_kernel_spmd(
        nc, in_maps, list(range(NCORES)), trace=_want_trace
    )

    # ---- unshard ----
    out_nb = np.empty((N * B, OUT), dtype=np.float32)
    for c in range(C):
        yt_packed = res.results[c]["yt"].astype(np.float32)  # [128, OC*rows] packed
        n_real = len(atom_ids[c]) * B
        for off, bs in blocks:
            if off >= n_real:
                break
            take = min(bs, n_real - off)
            blk = (
                yt_packed[:, OC * off : OC * (off + bs)]
                .reshape(P, OC, bs)
                .transpose(1, 0, 2)
                .reshape(OUT, bs)
            )
            out_nb[core_cols[c][off : off + take]] = blk[:, :take].T
    out = np.ascontiguousarray(out_nb.reshape(N, B, OUT).transpose(1, 0, 2))

    if _want_trace:
        kernel._last_results = res
    return (out, ch_in)


# revision 6
# speedup vs baseline: 2.0833x; 1.0951x over previous
"""Self-contained Trainium2 Bass kernel for the routed-dense (MoE-style) layer.

Reference computation (per batch b, atom n):
    out[b,n,:] = tanh(W[ch[n]] @ x[b,n,:] + bias[ch[n]]) + x[b,n,:]
    returns (out, channels)

Strategy: expert-parallel — core c owns channel c outright (C == n_cores == 8),
so each core loads exactly one [OUT, IN] weight.  Rows (b, n) are routed to
cores by channel on the host; every core is padded to the max channel's row
count so all 8 cores run the identical SPMD program.  On device everything
lives in transposed layout out^T[o, row]: the x^T tiles feed both the matmul
(contraction over IN on partitions) and the residual add (IN == OUT).
Host-side x / out streams are block-packed so each block moves with a single
DMA whose per-partition runs are contiguous.

Two precision modes (MODE below):
  "fp16": x/w/out in fp16, fp32 PSUM accumulation (fastest, rel err ~5e-4)
  "f32r": x/w fp32 with float32r matmuls, fp16 out (rel err ~2.4e-4)
"""

import sys

for _p in ("/opt/trn_rl_repo", "/root/.axon_site/_ro/trn_rl_repo"):
    if _p not in sys.path:
        sys.path.insert(0, _p)

import numpy as np

MODE = "fp16"

B, N, IN, OUT, C = 64, 1024, 512, 512, 8
NCORES = 8
P = 128
KC = IN // P   # 4 contraction chunks
OC = OUT // P  # 4 output-partition chunks

_cache = {}


def _plan_blocks(rows):
    """Chunk `rows` columns into blocks <= 512, preferring >= 256 so fp32r
    matmuls run at full rate."""
    blocks = []
    off = 0
    rem = rows
    while rem > 0:
        if rem > 768:
            take = 512
        elif rem > 512:
            take = (rem // 2) + (rem & 1)
        else:
            take = rem
        blocks.append((off, take))
        off += take
        rem -= take
    return blocks


def _build_program(rows, mode):
    import concourse.bacc as bacc
    import concourse.tile as tile
    import concourse.mybir as mybir
    from contextlib import ExitStack

    F32 = mybir.dt.float32
    F16 = mybir.dt.float16
    F32R = mybir.dt.float32r
    MM_DT = F16 if mode == "fp16" else F32R
    RES_DT = F16 if mode == "fp16" else F32

    blocks = _plan_blocks(rows)

    nc = bacc.Bacc("TRN2", target_bir_lowering=False, debug=False)
    # block-packed x^T: [128, 4 * rows], block b occupying [:, 4*off : 4*(off+bs)]
    # with inner layout [kc][col]
    xt_ext = nc.dram_tensor("xt", [P, KC * rows], RES_DT, kind="ExternalInput")
    # weight^T for this core's channel, packed [128, KC * OUT] ([kc][o])
    wt_ext = nc.dram_tensor("wt", [P, KC * OUT], RES_DT, kind="ExternalInput")
    b_ext = nc.dram_tensor("b", [P, OC], F32, kind="ExternalInput")
    # block-packed out^T: [128, 4 * rows], inner layout [oc][col]
    yt_ext = nc.dram_tensor("yt", [P, OC * rows], F16, kind="ExternalOutput")

    with tile.TileContext(nc) as tc:
        with ExitStack() as ctx:
            wpool = ctx.enter_context(tc.tile_pool(name="w", bufs=1))
            bpool = ctx.enter_context(tc.tile_pool(name="bias", bufs=1))
            xpool = ctx.enter_context(tc.tile_pool(name="x", bufs=9))
            apool = ctx.enter_context(tc.tile_pool(name="a", bufs=8))
            opool = ctx.enter_context(tc.tile_pool(name="o", bufs=8))
            ppool = ctx.enter_context(tc.tile_pool(name="p", bufs=8, space="PSUM"))

            w_sb = wpool.tile([P, KC * OUT], MM_DT)
            nc.sync.dma_start(w_sb[:], wt_ext[:].bitcast(MM_DT))
            b_sb = bpool.tile([P, OC], F32)
            nc.sync.dma_start(b_sb[:], b_ext[:])

            for off, bs in blocks:
                xt = xpool.tile([P, KC * 512], MM_DT, tag="xt")
                nc.sync.dma_start(
                    xt[:, : KC * bs],
                    xt_ext[:, KC * off : KC * (off + bs)].bitcast(MM_DT),
                )
                o_sb = opool.tile([P, OC * 512], F16, tag="out")
                for oc in range(OC):
                    psum = ppool.tile([P, 512], F32, tag="ps")
                    for kc in range(KC):
                        nc.tensor.matmul(
                            psum[:, :bs],
                            lhsT=w_sb[:, kc * OUT + oc * P : kc * OUT + (oc + 1) * P],
                            rhs=xt[:, kc * bs : (kc + 1) * bs],
                            start=(kc == 0),
                            stop=(kc == KC - 1),
                        )
                    a_sb = apool.tile([P, 512], F16, tag="act")
                    nc.scalar.activation(
                        a_sb[:, :bs],
                        psum[:, :bs],
                        mybir.ActivationFunctionType.Tanh,
                        bias=b_sb[:, oc : oc + 1],
                    )
                    nc.vector.tensor_add(
                        o_sb[:, oc * bs : (oc + 1) * bs],
                        a_sb[:, :bs],
                        xt[:, oc * bs : (oc + 1) * bs].bitcast(RES_DT),
                    )
                nc.scalar.dma_start(
                    yt_ext[:, OC * off : OC * (off + bs)], o_sb[:, : OC * bs]
                )

    nc.compile()
    return nc, blocks


def _get_program(rows, mode):
    key = (rows, mode)
    if key not in _cache:
        _cache[key] = _build_program(rows, mode)
    return _cache[key]


def kernel(x, channels, weight, bias, _want_trace=False):
    from concourse.bass_utils import run_bass_kernel_spmd

    x = np.asarray(x)
    ch_in = channels
    ch = np.asarray(channels).astype(np.int64)
    weight = np.asarray(weight, dtype=np.float32)
    bias = np.asarray(bias, dtype=np.float32)

    in_np_dt = np.float16 if MODE == "fp16" else np.float32

    counts = np.bincount(ch, minlength=C)
    rows = int(counts.max()) * B
    nc, blocks = _get_program(rows, MODE)

    # ---- host-side routing / sharding ----
    # column order per core: this core's channel's atoms, batch innermost,
    # padded to `rows` by repeating the first column
    atom_ids = [np.where(ch == c)[0] for c in range(C)]
    x_nb = np.ascontiguousarray(x.transpose(2, 1, 0)).reshape(IN, N * B)
    if MODE == "fp16":
        x_nb = x_nb.astype(np.float16)

    core_cols = []
    for c in range(C):
        cols = (atom_ids[c][:, None] * B + np.arange(B)[None, :]).ravel()
        if len(cols) < rows:
            pad = np.zeros(rows - len(cols), dtype=np.int64)
            if len(cols):
                pad[:] = cols[0]
            cols = np.concatenate([cols, pad])
        core_cols.append(cols)

    in_maps = []
    for c in range(C):
        xt = x_nb[:, core_cols[c]]  # [IN, rows]
        # block-pack: [128, 4 * rows], block b -> [:, 4*off:4*(off+bs)] = [kc][col]
        xt_packed = np.empty((P, KC * rows), dtype=in_np_dt)
        xt4 = xt.reshape(KC, P, rows)
        for off, bs in blocks:
            xt_packed[:, KC * off : KC * (off + bs)] = (
                xt4[:, :, off : off + bs].transpose(1, 0, 2).reshape(P, KC * bs)
            )
        wt = np.ascontiguousarray(
            weight[c].T.reshape(KC, P, OUT).transpose(1, 0, 2).reshape(P, KC * OUT)
        ).astype(in_np_dt)
        b_in = np.ascontiguousarray(bias[c].reshape(OC, P).T)
        in_maps.append({"xt": xt_packed, "wt": wt, "b": b_in})

    res = run_bass_kernel_spmd(
        nc, in_maps, list(range(NCORES)), trace=_want_trace
    )

    # ---- unshard ----
    out_nb = np.empty((N * B, OUT), dtype=np.float32)
    for c in range(C):
        yt_packed = res.results[c]["yt"].astype(np.float32)  # [128, OC*rows] packed
        n_real = len(atom_ids[c]) * B
        for off, bs in blocks:
            if off >= n_real:
                break
            take = min(bs, n_real - off)
            blk = (
                yt_packed[:, OC * off : OC * (off + bs)]
                .reshape(P, OC, bs)
                .transpose(1, 0, 2)
                .reshape(OUT, bs)
            )
            out_nb[core_cols[c][off : off + take]] = blk[:, :take].T
    out = np.ascontiguousarray(out_nb.reshape(N, B, OUT).transpose(1, 0, 2))

    if _want_trace:
        kernel._last_results = res
    return (out, ch_in)


# revision 7
# speedup vs baseline: 2.0861x; 1.0013x over previous
"""Self-contained Trainium2 Bass kernel for the routed-dense (MoE-style) layer.

Reference computation (per batch b, atom n):
    out[b,n,:] = tanh(W[ch[n]] @ x[b,n,:] + bias[ch[n]]) + x[b,n,:]
    returns (out, channels)

Strategy: expert-parallel — core c owns channel c outright (C == n_cores == 8),
so each core loads exactly one [OUT, IN] weight.  Rows (b, n) are routed to
cores by channel on the host; every core is padded to the max channel's row
count so all 8 cores run the identical SPMD program.  On device everything
lives in transposed layout out^T[o, row]: the x^T tiles feed both the matmul
(contraction over IN on partitions) and the residual add (IN == OUT).
Host-side x / out streams are block-packed so each block moves with a single
DMA whose per-partition runs are contiguous.

Two precision modes (MODE below):
  "fp16": x/w/out in fp16, fp32 PSUM accumulation (fastest, rel err ~5e-4)
  "f32r": x/w fp32 with float32r matmuls, fp16 out (rel err ~2.4e-4)
"""

import sys

for _p in ("/opt/trn_rl_repo", "/root/.axon_site/_ro/trn_rl_repo"):
    if _p not in sys.path:
        sys.path.insert(0, _p)

import numpy as np

MODE = "fp16"

B, N, IN, OUT, C = 64, 1024, 512, 512, 8
NCORES = 8
P = 128
KC = IN // P   # 4 contraction chunks
OC = OUT // P  # 4 output-partition chunks

_cache = {}


def _plan_blocks(rows):
    """Chunk `rows` columns into blocks <= 512, preferring >= 256 so fp32r
    matmuls run at full rate."""
    blocks = []
    off = 0
    rem = rows
    while rem > 0:
        if rem > 768:
            take = 512
        elif rem > 512:
            take = (rem // 2) + (rem & 1)
        else:
            take = rem
        blocks.append((off, take))
        off += take
        rem -= take
    return blocks


def _build_program(rows, mode):
    import concourse.bacc as bacc
    import concourse.tile as tile
    import concourse.mybir as mybir
    from contextlib import ExitStack

    F32 = mybir.dt.float32
    F16 = mybir.dt.float16
    F32R = mybir.dt.float32r
    MM_DT = F16 if mode == "fp16" else F32R
    RES_DT = F16 if mode == "fp16" else F32

    blocks = _plan_blocks(rows)

    nc = bacc.Bacc("TRN2", target_bir_lowering=False, debug=False)
    # block-packed x^T: [128, 4 * rows], block b occupying [:, 4*off : 4*(off+bs)]
    # with inner layout [kc][col]
    xt_ext = nc.dram_tensor("xt", [P, KC * rows], RES_DT, kind="ExternalInput")
    # weight^T for this core's channel, packed [128, KC * OUT] ([kc][o])
    wt_ext = nc.dram_tensor("wt", [P, KC * OUT], RES_DT, kind="ExternalInput")
    b_ext = nc.dram_tensor("b", [P, OC], F32, kind="ExternalInput")
    # block-packed out^T: [128, 4 * rows], inner layout [oc][col]
    yt_ext = nc.dram_tensor("yt", [P, OC * rows], F16, kind="ExternalOutput")

    with tile.TileContext(nc) as tc:
        with ExitStack() as ctx:
            wpool = ctx.enter_context(tc.tile_pool(name="w", bufs=1))
            bpool = ctx.enter_context(tc.tile_pool(name="bias", bufs=1))
            xpool = ctx.enter_context(tc.tile_pool(name="x", bufs=12))
            apool = ctx.enter_context(tc.tile_pool(name="a", bufs=8))
            opool = ctx.enter_context(tc.tile_pool(name="o", bufs=8))
            ppool = ctx.enter_context(tc.tile_pool(name="p", bufs=8, space="PSUM"))

            w_sb = wpool.tile([P, KC * OUT], MM_DT)
            nc.sync.dma_start(w_sb[:], wt_ext[:].bitcast(MM_DT))
            b_sb = bpool.tile([P, OC], F32)
            nc.sync.dma_start(b_sb[:], b_ext[:])

            for off, bs in blocks:
                xt = xpool.tile([P, KC * 512], MM_DT, tag="xt")
                nc.sync.dma_start(
                    xt[:, : KC * bs],
                    xt_ext[:, KC * off : KC * (off + bs)].bitcast(MM_DT),
                )
                o_sb = opool.tile([P, OC * 512], F16, tag="out")
                for oc in range(OC):
                    psum = ppool.tile([P, 512], F32, tag="ps")
                    for kc in range(KC):
                        nc.tensor.matmul(
                            psum[:, :bs],
                            lhsT=w_sb[:, kc * OUT + oc * P : kc * OUT + (oc + 1) * P],
                            rhs=xt[:, kc * bs : (kc + 1) * bs],
                            start=(kc == 0),
                            stop=(kc == KC - 1),
                        )
                    a_sb = apool.tile([P, 512], F16, tag="act")
                    nc.scalar.activation(
                        a_sb[:, :bs],
                        psum[:, :bs],
                        mybir.ActivationFunctionType.Tanh,
                        bias=b_sb[:, oc : oc + 1],
                    )
                    nc.vector.tensor_add(
                        o_sb[:, oc * bs : (oc + 1) * bs],
                        a_sb[:, :bs],
                        xt[:, oc * bs : (oc + 1) * bs].bitcast(RES_DT),
                    )
                nc.gpsimd.dma_start(
                    yt_ext[:, OC * off : OC * (off + bs)], o_sb[:, : OC * bs]
                )

    nc.compile()
    return nc, blocks


def _get_program(rows, mode):
    key = (rows, mode)
    if key not in _cache:
        _cache[key] = _build_program(rows, mode)
    return _cache[key]


def kernel(x, channels, weight, bias, _want_trace=False):
    from concourse.bass_utils import run_bass_kernel_spmd

    x = np.asarray(x)
    ch_in = channels
    ch = np.asarray(channels).astype(np.int64)
    weight = np.asarray(weight, dtype=np.float32)
    bias = np.asarray(bias, dtype=np.float32)

    in_np_dt = np.float16 if MODE == "fp16" else np.float32

    counts = np.bincount(ch, minlength=C)
    rows = int(counts.max()) * B
    nc, blocks = _get_program(rows, MODE)

    # ---- host-side routing / sharding ----
    # column order per core: this core's channel's atoms, batch innermost,
    # padded to `rows` by repeating the first column
    atom_ids = [np.where(ch == c)[0] for c in range(C)]
    x_nb = np.ascontiguousarray(x.transpose(2, 1, 0)).reshape(IN, N * B)
    if MODE == "fp16":
        x_nb = x_nb.astype(np.float16)

    core_cols = []
    for c in range(C):
        cols = (atom_ids[c][:, None] * B + np.arange(B)[None, :]).ravel()
        if len(cols) < rows:
            pad = np.zeros(rows - len(cols), dtype=np.int64)
            if len(cols):
                pad[:] = cols[0]
            cols = np.concatenate([cols, pad])
        core_cols.append(cols)

    in_maps = []
    for c in range(C):
        xt = x_nb[:, core_cols[c]]  # [IN, rows]
        # block-pack: [128, 4 * rows], block b -> [:, 4*off:4*(off+bs)] = [kc][col]
        xt_packed = np.empty((P, KC * rows), dtype=in_np_dt)
        xt4 = xt.reshape(KC, P, rows)
        for off, bs in blocks:
            xt_packed[:, KC * off : KC * (off + bs)] = (
                xt4[:, :, off : off + bs].transpose(1, 0, 2).reshape(P, KC * bs)
            )
        wt = np.ascontiguousarray(
            weight[c].T.reshape(KC, P, OUT).transpose(1, 0, 2).reshape(P, KC * OUT)
        ).astype(in_np_dt)
        b_in = np.ascontiguousarray(bias[c].reshape(OC, P).T)
        in_maps.append({"xt": xt_packed, "wt": wt, "b": b_in})

    res = run_bass_kernel_spmd(
        nc, in_maps, list(range(NCORES)), trace=_want_trace
    )

    # ---- unshard ----
    out_nb = np.empty((N * B, OUT), dtype=np.float32)
    for c in range(C):
        yt_packed = res.results[c]["yt"].astype(np.float32)  # [128, OC*rows] packed
        n_real = len(atom_ids[c]) * B
        for off, bs in blocks:
            if off >= n_real:
                break
            take = min(bs, n_real - off)
            blk = (
                yt_packed[:, OC * off : OC * (off + bs)]
                .reshape(P, OC, bs)
                .transpose(1, 0, 2)
                .reshape(OUT, bs)
            )
            out_nb[core_cols[c][off : off + take]] = blk[:, :take].T
    out = np.ascontiguousarray(out_nb.reshape(N, B, OUT).transpose(1, 0, 2))

    if _want_trace:
        kernel._last_results = res
    return (out, ch_in)
